# revision 32
# baseline (speedup 1.0000x reference)
"""Ensemble MLP surrogate (16 models, 32->64->64->64->8, relu) on 8 TRN2 cores.

Data-parallel over batch, weights replicated.  Feature-on-partition layout,
batch streamed as the matmul moving operand in fp16.  Per double-tile of 1024
batch elements the schedule is epilogue-bound (PSUM->SBUF bias+ReLU on the
Scalar/Vector engines), so the design centers on keeping those two engines
saturated:

  - L1 runs as 8-MM octets (two model-pairs concurrently, all 16 PE array
    tiles busy) via parity-swapped W1 packing for odd pairs; L2/L3 as 4-MM
    quads; L4 packs pair blockdiags into col-tiled preds banks.
  - PSUM (8 banks): php pool bufs=3 x [128,2,512] (6 banks; 3-deep rotation
    so matmul fills overlap drains) + p4 pool bufs=2 x [128,512] (2 banks).
    The ensemble mean / sum-of-squares selector matmuls are overlaid into
    unused partition rows (16-23 / 48-55) of the group-1 preds tiles, so the
    stats need no PSUM of their own.
  - Squares (pred^2, mean^2) run on the otherwise-idle GPSIMD engine; the
    E[p^2]-mean^2 subtraction is a -identity accumulating matmul on the PE.
  - Bias+ReLU epilogues are greedily balanced across Scalar and Vector.
  - Input DMA on sync (HWDGE), output DMA on scalar (HWDGE).
"""

import numpy as np

N_MODELS = 16
IN_DIM = 32
HID = 64
OUT_DIM = 8
BATCH = 131072
N_CORES = 8
B_CORE = BATCH // N_CORES  # 16384
TILE = 512  # PSUM bank limit on matmul out cols (fp32)
DTILE = 2 * TILE  # batch elements per pipeline step
NPAIR = N_MODELS // 2

# wpackr free-dim layout (fp16 matmul operands, 128 partitions)
OFF_W1 = 0  # [128, 8, 64]  odd pairs parity-swapped (rows b,a,b,a)
OFF_W2 = OFF_W1 + NPAIR * 64  # [128, 8, 64] canonical (a; b)
OFF_W3 = OFF_W2 + NPAIR * 64  # [128, 8, 64] odd pairs input-swapped (b; a)
OFF_W4 = OFF_W3 + NPAIR * 64  # [128, 8, 32] pair blockdiag (cols 0-15 zero)
OFF_SELP = OFF_W4 + NPAIR * 32  # [128, 8] mean/sumsq selector (1/16)
OFF_NEGD = OFF_SELP + 8  # [128, 8] -identity (rows 0-7)
WR = OFF_NEGD + 8
# wpackb free-dim layout (fp32 biases)
OFF_B1 = 0  # [128, 8]
OFF_B2 = OFF_B1 + NPAIR  # [128, 8] (parity-swapped odd pairs)
OFF_B3 = OFF_B2 + NPAIR  # [128, 8]
OFF_B4 = OFF_B3 + NPAIR  # [128, 2] (per L4 group)
WB = OFF_B4 + 2

USE_GPS_SQ = True  # pred squares on GPSIMD (else Vector)
USE_GPS_M2 = True  # mean^2 on GPSIMD (else Vector)


# epilogue op costs (ns) for greedy ACT/DVE load balancing (HW-measured)
def _act_cost(fd):
    return (315 + fd) / 1.2


def _dve_cost(fd):
    return (208 + fd) / 0.96


def pack_inputs(x, W1, b1, W2, b2, W3, b3, W4, b4, b_core=B_CORE, n_cores=N_CORES):
    """Host-side packing. Returns (xt_per_core list, wpackr fp16, wpackb f32)."""
    f32 = np.float32
    x = np.ascontiguousarray(x, dtype=f32)
    wpack = np.zeros((128, WR), f32)
    wpackb = np.zeros((128, WB), f32)

    w1v = wpack[:, OFF_W1 : OFF_W1 + NPAIR * 64].reshape(128, NPAIR, 64)
    w2v = wpack[:, OFF_W2 : OFF_W2 + NPAIR * 64].reshape(128, NPAIR, 64)
    w3v = wpack[:, OFF_W3 : OFF_W3 + NPAIR * 64].reshape(128, NPAIR, 64)
    w4v = wpack[:, OFF_W4 : OFF_W4 + NPAIR * 32].reshape(128, NPAIR, 32)
    for j in range(NPAIR):
        a, b = 2 * j, 2 * j + 1
        for k in range(4):
            # L1 row-group k: even pairs rows a,b,a,b; odd pairs b,a,b,a so the
            # swapped-col octet positions still land both outputs canonically
            if j % 2 == 0:
                w1v[32 * k : 32 * k + 32, j, :] = W1[a if k % 2 == 0 else b]
            else:
                w1v[32 * k : 32 * k + 32, j, :] = W1[b if k % 2 == 0 else a]
        w2v[0:HID, j, :] = W2[a]
        w2v[HID:128, j, :] = W2[b]
        if j % 2 == 0:  # h2 canonical input
            w3v[0:HID, j, :] = W3[a]
            w3v[HID:128, j, :] = W3[b]
        else:  # h2 swapped input (slot0 = model b)
            w3v[0:HID, j, :] = W3[b]
            w3v[HID:128, j, :] = W3[a]
        # preds land at partitions 32q+16..32q+32, freeing 32-aligned rows
        # 32q..32q+16 for the stats overlay
        w4v[0:HID, j, 16 : 16 + OUT_DIM] = W4[a]
        w4v[HID:128, j, 16 + OUT_DIM : 32] = W4[b]

    selp = wpack[:, OFF_SELP : OFF_SELP + 8]
    negd = wpack[:, OFF_NEGD : OFF_NEGD + 8]
    b4v = wpackb[:, OFF_B4 : OFF_B4 + 2]
    for q in range(4):  # pair-within-group
        for c in range(2):  # model-within-pair
            for o in range(OUT_DIM):
                p = 32 * q + 16 + 8 * c + o
                selp[p, o] = 1.0 / 16.0  # exact in fp16
                b4v[p, 0] = b4[2 * q + c, o]  # group 0: pairs 0-3
                b4v[p, 1] = b4[2 * (q + 4) + c, o]  # group 1: pairs 4-7
    for o in range(OUT_DIM):
        negd[o, o] = -1.0
    for j in range(NPAIR):
        a, b = 2 * j, 2 * j + 1
        wpackb[0:HID, OFF_B1 + j] = b1[a]
        wpackb[HID:128, OFF_B1 + j] = b1[b]
        # h2 output of odd pairs is parity-swapped (slot0 = model b)
        wpackb[0:HID, OFF_B2 + j] = b2[a if j % 2 == 0 else b]
        wpackb[HID:128, OFF_B2 + j] = b2[b if j % 2 == 0 else a]
        wpackb[0:HID, OFF_B3 + j] = b3[a]
        wpackb[HID:128, OFF_B3 + j] = b3[b]

    wpack16 = wpack.astype(np.float16)
    x16 = x.astype(np.float16)
    xt_per_core = []
    for c in range(n_cores):
        shard = x16[c * b_core : (c + 1) * b_core]  # [b_core, 32]
        xt = np.ascontiguousarray(np.tile(shard.T, (4, 1)))  # [128, b_core]
        xt_per_core.append(xt)
    return xt_per_core, wpack16, wpackb


def _emit(tc, ctx, xt, wr, wb, meant, stdt, b_core):
    import concourse.bass as bass  # noqa: F401
    from concourse import mybir

    nc = tc.nc
    f32 = mybir.dt.float32
    f16 = mybir.dt.float16
    AF = mybir.ActivationFunctionType
    ALU = mybir.AluOpType

    n_dt = b_core // DTILE

    consts = ctx.enter_context(tc.tile_pool(name="consts", bufs=1))
    xp = ctx.enter_context(tc.tile_pool(name="xp", bufs=3))
    hp = [
        ctx.enter_context(tc.tile_pool(name=f"h{i}p", bufs=8)) for i in range(3)
    ]
    prp = ctx.enter_context(tc.tile_pool(name="prp", bufs=4))
    sqp = ctx.enter_context(tc.tile_pool(name="sqp", bufs=4))
    msp = ctx.enter_context(tc.tile_pool(name="msp", bufs=2))  # mean/std/m2 sbuf
    # PSUM budget (8 banks): one pool, php 4x[128,2,512].  Every tile
    # (layers, preds, stats accumulators) is short-lived, so the 4-deep
    # rotation fully hides matmul fills behind the two engines' drains.
    php = ctx.enter_context(tc.tile_pool(name="php", bufs=4, space="PSUM"))

    cw = consts.tile([128, WR], f16)
    nc.sync.dma_start(out=cw, in_=wr)
    cwb = consts.tile([128, WB], f32)
    nc.sync.dma_start(out=cwb, in_=wb)
    # tiny warmup activations: trigger both ACT table-set loads at kernel
    # start (overlapped with the const/input DMAs) instead of stalling the
    # first real epilogue ~2.6us
    warm = consts.tile([128, 2], f32, tag="warm")
    nc.scalar.activation(warm[:, 0:1], cwb[:, 0:1], AF.Relu)
    nc.scalar.activation(warm[:, 1:2], cwb[:, 0:1], AF.Sqrt)
    w1v = cw[:, OFF_W1 : OFF_W1 + NPAIR * 64].rearrange("p (j f) -> p j f", f=64)
    w2v = cw[:, OFF_W2 : OFF_W2 + NPAIR * 64].rearrange("p (j f) -> p j f", f=64)
    w3v = cw[:, OFF_W3 : OFF_W3 + NPAIR * 64].rearrange("p (j f) -> p j f", f=64)
    w4v = cw[:, OFF_W4 : OFF_W4 + NPAIR * 32].rearrange("p (j f) -> p j f", f=32)
    selp = cw[:, OFF_SELP : OFF_SELP + 8]
    negd = cw[0:8, OFF_NEGD : OFF_NEGD + 8]

    # greedy engine balancer for PSUM->SBUF epilogues
    eng_ns = {"act": 0.0, "dve": 0.0}

    def epilogue(out, in_, bias, relu):
        fd = out.free_size()
        if eng_ns["act"] + _act_cost(fd) <= eng_ns["dve"] + _dve_cost(fd):
            eng_ns["act"] += _act_cost(fd)
            nc.scalar.activation(
                out, in_, AF.Relu if relu else AF.Identity, bias=bias, scale=1.0
            )
        else:
            eng_ns["dve"] += _dve_cost(fd)
            if relu:
                nc.vector.tensor_scalar(
                    out, in_, bias, 0.0, op0=ALU.add, op1=ALU.max
                )
            else:
                nc.vector.tensor_scalar(out, in_, bias, None, op0=ALU.add)

    def l1_octet(d, xt_t, ph):
        j0, j1 = 2 * d, 2 * d + 1
        for k in range(4):
            rg = 32 * k
            c0 = 64 * (k % 2)  # even pair col half
            c1 = 64 * ((k + 1) % 2)  # odd pair col half (swapped)
            nc.tensor.matmul(
                out=ph[j0][c0 : c0 + 64, k // 2, :],
                lhsT=w1v[rg : rg + 32, j0, :],
                rhs=xt_t[rg : rg + 32, k // 2, :],
                start=True, stop=True, tile_position=(rg, c0),
            )
            nc.tensor.matmul(
                out=ph[j1][c1 : c1 + 64, k // 2, :],
                lhsT=w1v[rg : rg + 32, j1, :],
                rhs=xt_t[rg : rg + 32, k // 2, :],
                start=True, stop=True, tile_position=(rg, c1),
            )

    def mid_quads(d, wv, hin, ph):
        j0, j1 = 2 * d, 2 * d + 1
        for h in range(2):
            nc.tensor.matmul(
                out=ph[j0][0:64, h, :], lhsT=wv[0:64, j0, :],
                rhs=hin[j0][0:64, h, :], start=True, stop=True,
                tile_position=(0, 0),
            )
            nc.tensor.matmul(
                out=ph[j0][64:128, h, :], lhsT=wv[64:128, j0, :],
                rhs=hin[j0][64:128, h, :], start=True, stop=True,
                tile_position=(64, 64),
            )
            nc.tensor.matmul(
                out=ph[j1][64:128, h, :], lhsT=wv[0:64, j1, :],
                rhs=hin[j1][0:64, h, :], start=True, stop=True,
                tile_position=(0, 64),
            )
            nc.tensor.matmul(
                out=ph[j1][0:64, h, :], lhsT=wv[64:128, j1, :],
                rhs=hin[j1][64:128, h, :], start=True, stop=True,
                tile_position=(64, 0),
            )

    def layer_duo(lnum, t, xt_t, hin, hout, d):
        """One duo (2 pairs) of one layer: an 8-MM PE burst + 2 epilogues."""
        wv = (None, w2v, w3v)[lnum - 1]
        boff = (OFF_B1, OFF_B2, OFF_B3)[lnum - 1]
        j0, j1 = 2 * d, 2 * d + 1
        ph = {}
        for j in (j0, j1):
            ph[j] = php.tile(
                [128, 2, TILE], f32, tag="ph", name=f"ph{lnum}_{t}_{j}"
            )
        if lnum == 1:
            l1_octet(d, xt_t, ph)
        else:
            mid_quads(d, wv, hin, ph)
        for j in (j0, j1):
            hout[j] = hp[lnum - 1].tile(
                [128, 2, TILE], f16, tag=f"h{lnum}", name=f"h{lnum}_{t}_{j}"
            )
            epilogue(
                hout[j].rearrange("p h n -> p (h n)"),
                ph[j].rearrange("p h n -> p (h n)"),
                cwb[:, boff + j : boff + j + 1],
                relu=True,
            )

    def l4_block(g, pend):
        """Deferred L4 for group g of the previous dtile (deps long resolved)."""
        h3 = pend["h3"]
        p4g = php.tile([128, 2, TILE], f32, tag="ph", name=f"p4_{g}")
        pend["p4"][g] = p4g
        for d in (2 * g, 2 * g + 1):
            for h in range(2):
                for j in (2 * d, 2 * d + 1):
                    q = j % 4
                    nc.tensor.matmul(
                        out=p4g[32 * q : 32 * q + 32, h, :],
                        lhsT=w4v[:, j, :],
                        rhs=h3[j][:, h, :],
                        start=True, stop=True,
                        tile_position=(0, 32 * q),
                    )
        prt = prp.tile([128, 2, TILE], f16, tag="pr", name=f"pr_{g}")
        epilogue(
            prt.rearrange("p h n -> p (h n)"),
            p4g.rearrange("p h n -> p (h n)"),
            cwb[:, OFF_B4 + g : OFF_B4 + g + 1],
            relu=False,
        )
        pend["pr"][g] = prt
        sqt = sqp.tile([128, 2, TILE], f16, tag="sq", name=f"sq_{g}")
        if USE_GPS_SQ:
            nc.gpsimd.tensor_mul(
                sqt.rearrange("p h n -> p (h n)"),
                prt.rearrange("p h n -> p (h n)"),
                prt.rearrange("p h n -> p (h n)"),
            )
        else:
            nc.vector.tensor_mul(
                sqt.rearrange("p h n -> p (h n)"),
                prt.rearrange("p h n -> p (h n)"),
                prt.rearrange("p h n -> p (h n)"),
            )
            eng_ns["dve"] += (58 + 2 * TILE) / 0.96
        pend["sq"][g] = sqt

    def stats_sel_mean(pend):
        """Ensemble mean selector MMs into a fresh short-lived rotation tile
        (partitions 0-7); depends only on pr (ready early), so it doubles as
        safe PE filler mid-L2-phase."""
        pr = pend["pr"]
        tgt = php.tile([128, 2, TILE], f32, tag="ph", name="mean_ps")
        pend["mean_ps"] = tgt
        for h in range(2):
            nc.tensor.matmul(
                out=tgt[0:8, h, :], lhsT=selp, rhs=pr[0][:, h, :],
                start=True, stop=False, tile_position=(0, 0),
            )
            nc.tensor.matmul(
                out=tgt[0:8, h, :], lhsT=selp, rhs=pr[1][:, h, :],
                start=False, stop=True, tile_position=(0, 0),
            )

    def stats_sel_sq(pend):
        """E[p^2] selector MMs into a fresh rotation tile, drained to SBUF
        immediately; deferred until the gpsimd squares have landed."""
        sq = pend["sq"]
        tgt = php.tile([128, 2, TILE], f32, tag="ph", name="sqe_ps")
        for h in range(2):
            nc.tensor.matmul(
                out=tgt[0:8, h, :], lhsT=selp, rhs=sq[0][:, h, :],
                start=True, stop=False, tile_position=(0, 0),
            )
            nc.tensor.matmul(
                out=tgt[0:8, h, :], lhsT=selp, rhs=sq[1][:, h, :],
                start=False, stop=True, tile_position=(0, 0),
            )
        sqe_sb = msp.tile([8, 2, TILE], f32, tag="sqe")
        epilogue(
            sqe_sb.rearrange("p h n -> p (h n)"),
            tgt[0:8, :, :].rearrange("p h n -> p (h n)"),
            0.0, relu=False,
        )
        pend["sqe_sb"] = sqe_sb

    def stats_copy(pend):
        mean_sb = msp.tile([8, 2, TILE], f32, tag="mean")
        epilogue(
            mean_sb.rearrange("p h n -> p (h n)"),
            pend["mean_ps"][0:8, :, :].rearrange("p h n -> p (h n)"),
            0.0, relu=False,
        )
        pend["mean_sb"] = mean_sb

    def stats_m2(pend):
        mean_sb = pend["mean_sb"]
        m2t = msp.tile([8, 2, TILE], f32, tag="m2")
        if USE_GPS_M2:
            nc.gpsimd.tensor_mul(
                m2t.rearrange("p h n -> p (h n)"),
                mean_sb.rearrange("p h n -> p (h n)"),
                mean_sb.rearrange("p h n -> p (h n)"),
            )
        else:
            nc.vector.tensor_mul(
                m2t.rearrange("p h n -> p (h n)"),
                mean_sb.rearrange("p h n -> p (h n)"),
                mean_sb.rearrange("p h n -> p (h n)"),
            )
            eng_ns["dve"] += (58 + 2 * TILE) / 0.96
        pend["m2t"] = m2t

    def stats_sub_sqrt(pend):
        """nvar = E[p^2] - mean^2 on gpsimd (SBUF-side; nothing holds PSUM),
        then std = sqrt(16/15 * nvar) on ACT."""
        nvar = msp.tile([8, 2, TILE], f32, tag="nvar")
        nc.gpsimd.tensor_sub(
            nvar.rearrange("p h n -> p (h n)"),
            pend["sqe_sb"].rearrange("p h n -> p (h n)"),
            pend["m2t"].rearrange("p h n -> p (h n)"),
        )
        std_sb = msp.tile([8, 2, TILE], f32, tag="std")
        nc.scalar.activation(
            out=std_sb.rearrange("p h n -> p (h n)"),
            in_=nvar.rearrange("p h n -> p (h n)"),
            func=AF.Sqrt, scale=16.0 / 15.0,
        )
        eng_ns["act"] += _act_cost(2 * TILE)
        pend["std_sb"] = std_sb

    def xt_fetch(t):
        x0 = t * DTILE
        tile = xp.tile([128, 2, TILE], f16, tag="xt", name=f"xt_{t}")
        nc.sync.dma_start(
            out=tile,
            in_=xt[:, x0 : x0 + DTILE].rearrange("p (h n) -> p h n", n=TILE),
        )
        return tile

    # Software pipeline with fine-grained interleaving: dtile t-1's L4,
    # preds, and stats-chain links are woven between dtile t's duo bursts.
    # Each deferred block's dependencies resolved >=2 phases earlier, so it
    # never stalls the PE FIFO head, and it gives the PE dependency-free
    # filler work during epilogue drain waits (keeps HAM warm).
    # Input tiles are prefetched 2 dtiles ahead (xp bufs=3) so L1 never
    # waits on the inbound DMA.
    xt_tiles = {0: xt_fetch(0), 1: xt_fetch(1)}
    pend = None
    for t in range(n_dt + 1):
        run = t < n_dt
        if run:
            x0 = t * DTILE
            if t + 2 < n_dt:
                xt_tiles[t + 2] = xt_fetch(t + 2)
            xt_t = xt_tiles.pop(t)
            h1, h2, h3 = {}, {}, {}
            layer_duo(1, t, xt_t, None, h1, 0)
        if pend is not None:
            l4_block(0, pend)
        if run:
            layer_duo(1, t, xt_t, None, h1, 1)
            layer_duo(1, t, xt_t, None, h1, 2)
            layer_duo(1, t, xt_t, None, h1, 3)
        if pend is not None:
            l4_block(1, pend)
        if run:
            layer_duo(2, t, None, h1, h2, 0)
            layer_duo(2, t, None, h1, h2, 1)
        if pend is not None:
            stats_sel_mean(pend)
        if run:
            layer_duo(2, t, None, h1, h2, 2)
        if pend is not None:
            stats_copy(pend)
        if run:
            layer_duo(2, t, None, h1, h2, 3)
        if pend is not None:
            # sq (gpsimd) has landed by now
            stats_sel_sq(pend)
            stats_m2(pend)
            nc.sync.dma_start(
                out=meant[:, pend["x0"] : pend["x0"] + DTILE].rearrange(
                    "p (h n) -> p h n", n=TILE
                ),
                in_=pend["mean_sb"],
            )
        if run:
            layer_duo(3, t, None, h2, h3, 0)
            layer_duo(3, t, None, h2, h3, 1)
            layer_duo(3, t, None, h2, h3, 2)
            layer_duo(3, t, None, h2, h3, 3)
        if pend is not None:
            stats_sub_sqrt(pend)
            nc.sync.dma_start(
                out=stdt[:, pend["x0"] : pend["x0"] + DTILE].rearrange(
                    "p (h n) -> p h n", n=TILE
                ),
                in_=pend["std_sb"],
            )
        pend = {"x0": x0, "h3": h3, "pr": {}, "sq": {}, "p4": {}} if run else None


def build(b_core=B_CORE, num_devices=N_CORES):
    from contextlib import ExitStack

    import concourse.bacc as bacc
    import concourse.tile as tile
    from concourse import mybir

    f32 = mybir.dt.float32
    f16 = mybir.dt.float16
    nc = bacc.Bacc(
        "TRN2", target_bir_lowering=False, debug=False, num_devices=num_devices
    )
    xt = nc.dram_tensor("xt", [128, b_core], f16, kind="ExternalInput").ap()
    wr = nc.dram_tensor("wpackr", [128, WR], f16, kind="ExternalInput").ap()
    wb = nc.dram_tensor("wpackb", [128, WB], f32, kind="ExternalInput").ap()
    meant = nc.dram_tensor("meant", [8, b_core], f32, kind="ExternalOutput").ap()
    stdt = nc.dram_tensor("stdt", [8, b_core], f32, kind="ExternalOutput").ap()
    with tile.TileContext(nc) as tc:
        with ExitStack() as ctx:
            _emit(tc, ctx, xt, wr, wb, meant, stdt, b_core)
    nc.compile()
    return nc


_NC_CACHE = {}


def kernel(x, W1, b1, W2, b2, W3, b3, W4, b4):
    from concourse.bass_utils import run_bass_kernel_spmd

    key = ("full", B_CORE)
    if key not in _NC_CACHE:
        _NC_CACHE[key] = build(B_CORE, N_CORES)
    nc = _NC_CACHE[key]

    xt_per_core, wpackr, wpackb = pack_inputs(
        np.asarray(x), np.asarray(W1), np.asarray(b1), np.asarray(W2),
        np.asarray(b2), np.asarray(W3), np.asarray(b3), np.asarray(W4),
        np.asarray(b4),
    )
    in_maps = [
        {"xt": xt_per_core[c], "wpackr": wpackr, "wpackb": wpackb}
        for c in range(N_CORES)
    ]
    res = run_bass_kernel_spmd(nc, in_maps, list(range(N_CORES))).results
    mean = np.concatenate([res[c]["meant"] for c in range(N_CORES)], axis=1).T
    std = np.concatenate([res[c]["stdt"] for c in range(N_CORES)], axis=1).T
    return np.ascontiguousarray(mean), np.ascontiguousarray(std)


# revision 36
# speedup vs baseline: 1.0188x; 1.0188x over previous
"""Ensemble MLP surrogate (16 models, 32->64->64->64->8, relu) on 8 TRN2 cores.

Data-parallel over batch, weights replicated.  Feature-on-partition layout,
batch streamed as the matmul moving operand in fp16.  Per double-tile of 1024
batch elements the schedule is epilogue-bound (PSUM->SBUF bias+ReLU on the
Scalar/Vector engines), so the design centers on keeping those two engines
saturated:

  - L1 runs as 8-MM octets (two model-pairs concurrently, all 16 PE array
    tiles busy) via parity-swapped W1 packing for odd pairs; L2/L3 as 4-MM
    quads; L4 packs pair blockdiags into col-tiled preds banks.
  - PSUM (8 banks): php pool bufs=3 x [128,2,512] (6 banks; 3-deep rotation
    so matmul fills overlap drains) + p4 pool bufs=2 x [128,512] (2 banks).
    The ensemble mean / sum-of-squares selector matmuls are overlaid into
    unused partition rows (16-23 / 48-55) of the group-1 preds tiles, so the
    stats need no PSUM of their own.
  - Squares (pred^2, mean^2) run on the otherwise-idle GPSIMD engine; the
    E[p^2]-mean^2 subtraction is a -identity accumulating matmul on the PE.
  - Bias+ReLU epilogues are greedily balanced across Scalar and Vector.
  - Input DMA on sync (HWDGE), output DMA on scalar (HWDGE).
"""

import numpy as np

N_MODELS = 16
IN_DIM = 32
HID = 64
OUT_DIM = 8
BATCH = 131072
N_CORES = 8
B_CORE = BATCH // N_CORES  # 16384
TILE = 512  # PSUM bank limit on matmul out cols (fp32)
DTILE = 2 * TILE  # batch elements per pipeline step
NPAIR = N_MODELS // 2

# wpackr free-dim layout (fp16 matmul operands, 128 partitions)
OFF_W1 = 0  # [128, 8, 64]  odd pairs parity-swapped (rows b,a,b,a)
OFF_W2 = OFF_W1 + NPAIR * 64  # [128, 8, 64] canonical (a; b)
OFF_W3 = OFF_W2 + NPAIR * 64  # [128, 8, 64] odd pairs input-swapped (b; a)
OFF_W4 = OFF_W3 + NPAIR * 64  # [128, 8, 32] pair blockdiag (cols 0-15 zero)
OFF_SELP = OFF_W4 + NPAIR * 32  # [128, 8] mean/sumsq selector (1/16)
OFF_NEGD = OFF_SELP + 8  # [128, 8] -identity (rows 0-7)
WR = OFF_NEGD + 8
# wpackb free-dim layout (fp32 biases)
OFF_B1 = 0  # [128, 8]
OFF_B2 = OFF_B1 + NPAIR  # [128, 8] (parity-swapped odd pairs)
OFF_B3 = OFF_B2 + NPAIR  # [128, 8]
OFF_B4 = OFF_B3 + NPAIR  # [128, 2] (per L4 group)
WB = OFF_B4 + 2

USE_GPS_SQ = True  # pred squares on GPSIMD (else Vector)
USE_GPS_M2 = True  # mean^2 on GPSIMD (else Vector)


# epilogue op costs (ns) for greedy ACT/DVE load balancing (HW-measured)
def _act_cost(fd):
    return (315 + fd) / 1.2


def _dve_cost(fd):
    return (208 + fd) / 0.96


def pack_inputs(x, W1, b1, W2, b2, W3, b3, W4, b4, b_core=B_CORE, n_cores=N_CORES):
    """Host-side packing. Returns (xt_per_core list, wpackr fp16, wpackb f32)."""
    f32 = np.float32
    x = np.ascontiguousarray(x, dtype=f32)
    wpack = np.zeros((128, WR), f32)
    wpackb = np.zeros((128, WB), f32)

    w1v = wpack[:, OFF_W1 : OFF_W1 + NPAIR * 64].reshape(128, NPAIR, 64)
    w2v = wpack[:, OFF_W2 : OFF_W2 + NPAIR * 64].reshape(128, NPAIR, 64)
    w3v = wpack[:, OFF_W3 : OFF_W3 + NPAIR * 64].reshape(128, NPAIR, 64)
    w4v = wpack[:, OFF_W4 : OFF_W4 + NPAIR * 32].reshape(128, NPAIR, 32)
    for j in range(NPAIR):
        a, b = 2 * j, 2 * j + 1
        for k in range(4):
            # L1 row-group k: even pairs rows a,b,a,b; odd pairs b,a,b,a so the
            # swapped-col octet positions still land both outputs canonically
            if j % 2 == 0:
                w1v[32 * k : 32 * k + 32, j, :] = W1[a if k % 2 == 0 else b]
            else:
                w1v[32 * k : 32 * k + 32, j, :] = W1[b if k % 2 == 0 else a]
        w2v[0:HID, j, :] = W2[a]
        w2v[HID:128, j, :] = W2[b]
        if j % 2 == 0:  # h2 canonical input
            w3v[0:HID, j, :] = W3[a]
            w3v[HID:128, j, :] = W3[b]
        else:  # h2 swapped input (slot0 = model b)
            w3v[0:HID, j, :] = W3[b]
            w3v[HID:128, j, :] = W3[a]
        # preds land at partitions 32q+16..32q+32, freeing 32-aligned rows
        # 32q..32q+16 for the stats overlay
        w4v[0:HID, j, 16 : 16 + OUT_DIM] = W4[a]
        w4v[HID:128, j, 16 + OUT_DIM : 32] = W4[b]

    selp = wpack[:, OFF_SELP : OFF_SELP + 8]
    negd = wpack[:, OFF_NEGD : OFF_NEGD + 8]
    b4v = wpackb[:, OFF_B4 : OFF_B4 + 2]
    for q in range(4):  # pair-within-group
        for c in range(2):  # model-within-pair
            for o in range(OUT_DIM):
                p = 32 * q + 16 + 8 * c + o
                selp[p, o] = 1.0 / 16.0  # exact in fp16
                b4v[p, 0] = b4[2 * q + c, o]  # group 0: pairs 0-3
                b4v[p, 1] = b4[2 * (q + 4) + c, o]  # group 1: pairs 4-7
    for o in range(OUT_DIM):
        negd[o, o] = -1.0
    for j in range(NPAIR):
        a, b = 2 * j, 2 * j + 1
        wpackb[0:HID, OFF_B1 + j] = b1[a]
        wpackb[HID:128, OFF_B1 + j] = b1[b]
        # h2 output of odd pairs is parity-swapped (slot0 = model b)
        wpackb[0:HID, OFF_B2 + j] = b2[a if j % 2 == 0 else b]
        wpackb[HID:128, OFF_B2 + j] = b2[b if j % 2 == 0 else a]
        wpackb[0:HID, OFF_B3 + j] = b3[a]
        wpackb[HID:128, OFF_B3 + j] = b3[b]

    wpack16 = wpack.astype(np.float16)
    x16 = x.astype(np.float16)
    xt_per_core = []
    for c in range(n_cores):
        shard = x16[c * b_core : (c + 1) * b_core]  # [b_core, 32]
        xt = np.ascontiguousarray(np.tile(shard.T, (4, 1)))  # [128, b_core]
        xt_per_core.append(xt)
    return xt_per_core, wpack16, wpackb


def _emit(tc, ctx, xt, wr, wb, meant, stdt, b_core):
    import concourse.bass as bass  # noqa: F401
    from concourse import mybir

    nc = tc.nc
    f32 = mybir.dt.float32
    f16 = mybir.dt.float16
    AF = mybir.ActivationFunctionType
    ALU = mybir.AluOpType

    n_dt = b_core // DTILE

    consts = ctx.enter_context(tc.tile_pool(name="consts", bufs=1))
    xp = ctx.enter_context(tc.tile_pool(name="xp", bufs=3))
    hp = [
        ctx.enter_context(tc.tile_pool(name=f"h{i}p", bufs=8)) for i in range(3)
    ]
    prp = ctx.enter_context(tc.tile_pool(name="prp", bufs=4))
    sqp = ctx.enter_context(tc.tile_pool(name="sqp", bufs=4))
    msp = ctx.enter_context(tc.tile_pool(name="msp", bufs=2))  # mean/std/m2 sbuf
    # PSUM budget (8 banks): php 3x[128,2,512]=6 + p4p 1x[128,2,512]=2.
    # (A 4-deep single pool was tried: the extra slack lets the PE sprint
    # then idle past the HAM window -> 52% cold-clock; 3-deep backpressure
    # paces the PE and keeps it warm.)
    php = ctx.enter_context(tc.tile_pool(name="php", bufs=3, space="PSUM"))
    p4p = ctx.enter_context(tc.tile_pool(name="p4p", bufs=1, space="PSUM"))

    cw = consts.tile([128, WR], f16)
    nc.sync.dma_start(out=cw, in_=wr)
    cwb = consts.tile([128, WB], f32)
    nc.sync.dma_start(out=cwb, in_=wb)
    # tiny warmup activations: trigger both ACT table-set loads at kernel
    # start (overlapped with the const/input DMAs) instead of stalling the
    # first real epilogue ~2.6us
    warm = consts.tile([128, 2], f32, tag="warm")
    nc.scalar.activation(warm[:, 0:1], cwb[:, 0:1], AF.Relu)
    nc.scalar.activation(warm[:, 1:2], cwb[:, 0:1], AF.Sqrt)
    w1v = cw[:, OFF_W1 : OFF_W1 + NPAIR * 64].rearrange("p (j f) -> p j f", f=64)
    w2v = cw[:, OFF_W2 : OFF_W2 + NPAIR * 64].rearrange("p (j f) -> p j f", f=64)
    w3v = cw[:, OFF_W3 : OFF_W3 + NPAIR * 64].rearrange("p (j f) -> p j f", f=64)
    w4v = cw[:, OFF_W4 : OFF_W4 + NPAIR * 32].rearrange("p (j f) -> p j f", f=32)
    selp = cw[:, OFF_SELP : OFF_SELP + 8]
    negd = cw[0:8, OFF_NEGD : OFF_NEGD + 8]

    # greedy engine balancer for PSUM->SBUF epilogues
    eng_ns = {"act": 0.0, "dve": 0.0}

    def epilogue(out, in_, bias, relu):
        fd = out.free_size()
        if eng_ns["act"] + _act_cost(fd) <= eng_ns["dve"] + _dve_cost(fd):
            eng_ns["act"] += _act_cost(fd)
            nc.scalar.activation(
                out, in_, AF.Relu if relu else AF.Identity, bias=bias, scale=1.0
            )
        else:
            eng_ns["dve"] += _dve_cost(fd)
            if relu:
                nc.vector.tensor_scalar(
                    out, in_, bias, 0.0, op0=ALU.add, op1=ALU.max
                )
            else:
                nc.vector.tensor_scalar(out, in_, bias, None, op0=ALU.add)

    def l1_octet(d, xt_t, ph):
        j0, j1 = 2 * d, 2 * d + 1
        for k in range(4):
            rg = 32 * k
            c0 = 64 * (k % 2)  # even pair col half
            c1 = 64 * ((k + 1) % 2)  # odd pair col half (swapped)
            nc.tensor.matmul(
                out=ph[j0][c0 : c0 + 64, k // 2, :],
                lhsT=w1v[rg : rg + 32, j0, :],
                rhs=xt_t[rg : rg + 32, k // 2, :],
                start=True, stop=True, tile_position=(rg, c0),
            )
            nc.tensor.matmul(
                out=ph[j1][c1 : c1 + 64, k // 2, :],
                lhsT=w1v[rg : rg + 32, j1, :],
                rhs=xt_t[rg : rg + 32, k // 2, :],
                start=True, stop=True, tile_position=(rg, c1),
            )

    def mid_quads(d, wv, hin, ph):
        j0, j1 = 2 * d, 2 * d + 1
        for h in range(2):
            nc.tensor.matmul(
                out=ph[j0][0:64, h, :], lhsT=wv[0:64, j0, :],
                rhs=hin[j0][0:64, h, :], start=True, stop=True,
                tile_position=(0, 0),
            )
            nc.tensor.matmul(
                out=ph[j0][64:128, h, :], lhsT=wv[64:128, j0, :],
                rhs=hin[j0][64:128, h, :], start=True, stop=True,
                tile_position=(64, 64),
            )
            nc.tensor.matmul(
                out=ph[j1][64:128, h, :], lhsT=wv[0:64, j1, :],
                rhs=hin[j1][0:64, h, :], start=True, stop=True,
                tile_position=(0, 64),
            )
            nc.tensor.matmul(
                out=ph[j1][0:64, h, :], lhsT=wv[64:128, j1, :],
                rhs=hin[j1][64:128, h, :], start=True, stop=True,
                tile_position=(64, 0),
            )

    def layer_duo(lnum, t, xt_t, hin, hout, d):
        """One duo (2 pairs) of one layer: an 8-MM PE burst + 2 epilogues."""
        wv = (None, w2v, w3v)[lnum - 1]
        boff = (OFF_B1, OFF_B2, OFF_B3)[lnum - 1]
        j0, j1 = 2 * d, 2 * d + 1
        ph = {}
        for j in (j0, j1):
            ph[j] = php.tile(
                [128, 2, TILE], f32, tag="ph", name=f"ph{lnum}_{t}_{j}"
            )
        if lnum == 1:
            l1_octet(d, xt_t, ph)
        else:
            mid_quads(d, wv, hin, ph)
        for j in (j0, j1):
            hout[j] = hp[lnum - 1].tile(
                [128, 2, TILE], f16, tag=f"h{lnum}", name=f"h{lnum}_{t}_{j}"
            )
            epilogue(
                hout[j].rearrange("p h n -> p (h n)"),
                ph[j].rearrange("p h n -> p (h n)"),
                cwb[:, boff + j : boff + j + 1],
                relu=True,
            )

    def l4_block(g, pend):
        """Deferred L4 for group g of the previous dtile (deps long resolved)."""
        h3 = pend["h3"]
        p4g = p4p.tile([128, 2, TILE], f32, tag="p4", name=f"p4_{g}")
        pend["p4"][g] = p4g
        for d in (2 * g, 2 * g + 1):
            for h in range(2):
                for j in (2 * d, 2 * d + 1):
                    q = j % 4
                    nc.tensor.matmul(
                        out=p4g[32 * q : 32 * q + 32, h, :],
                        lhsT=w4v[:, j, :],
                        rhs=h3[j][:, h, :],
                        start=True, stop=True,
                        tile_position=(0, 32 * q),
                    )
        prt = prp.tile([128, 2, TILE], f16, tag="pr", name=f"pr_{g}")
        epilogue(
            prt.rearrange("p h n -> p (h n)"),
            p4g.rearrange("p h n -> p (h n)"),
            cwb[:, OFF_B4 + g : OFF_B4 + g + 1],
            relu=False,
        )
        pend["pr"][g] = prt
        sqt = sqp.tile([128, 2, TILE], f16, tag="sq", name=f"sq_{g}")
        if USE_GPS_SQ:
            nc.gpsimd.tensor_mul(
                sqt.rearrange("p h n -> p (h n)"),
                prt.rearrange("p h n -> p (h n)"),
                prt.rearrange("p h n -> p (h n)"),
            )
        else:
            nc.vector.tensor_mul(
                sqt.rearrange("p h n -> p (h n)"),
                prt.rearrange("p h n -> p (h n)"),
                prt.rearrange("p h n -> p (h n)"),
            )
            eng_ns["dve"] += (58 + 2 * TILE) / 0.96
        pend["sq"][g] = sqt

    def stats_sel_mean(pend):
        """Ensemble mean selector MMs, overlaid into unused partition rows
        0-7 (col group 0) of p4[1]; depends only on pr (ready early), so it
        doubles as safe PE filler mid-L2-phase."""
        pr, tgt = pend["pr"], pend["p4"][1]
        for h in range(2):
            nc.tensor.matmul(
                out=tgt[0:8, h, :], lhsT=selp, rhs=pr[0][:, h, :],
                start=True, stop=False, tile_position=(0, 0),
            )
            nc.tensor.matmul(
                out=tgt[0:8, h, :], lhsT=selp, rhs=pr[1][:, h, :],
                start=False, stop=True, tile_position=(0, 0),
            )

    def stats_sel_sq(pend):
        """E[p^2] selector MMs at partitions 32-39 (col group 1); deferred
        until the gpsimd squares have landed."""
        sq, tgt = pend["sq"], pend["p4"][1]
        for h in range(2):
            nc.tensor.matmul(
                out=tgt[32:40, h, :], lhsT=selp, rhs=sq[0][:, h, :],
                start=True, stop=False, tile_position=(0, 32),
            )
            nc.tensor.matmul(
                out=tgt[32:40, h, :], lhsT=selp, rhs=sq[1][:, h, :],
                start=False, stop=False, tile_position=(0, 32),
            )

    def stats_copy(pend):
        mean_sb = msp.tile([8, 2, TILE], f32, tag="mean")
        epilogue(
            mean_sb.rearrange("p h n -> p (h n)"),
            pend["p4"][1][0:8, :, :].rearrange("p h n -> p (h n)"),
            0.0, relu=False,
        )
        pend["mean_sb"] = mean_sb

    def stats_m2(pend):
        mean_sb = pend["mean_sb"]
        m2t = msp.tile([8, 2, TILE], f16, tag="m2")
        if USE_GPS_M2:
            nc.gpsimd.tensor_mul(
                m2t.rearrange("p h n -> p (h n)"),
                mean_sb.rearrange("p h n -> p (h n)"),
                mean_sb.rearrange("p h n -> p (h n)"),
            )
        else:
            nc.vector.tensor_mul(
                m2t.rearrange("p h n -> p (h n)"),
                mean_sb.rearrange("p h n -> p (h n)"),
                mean_sb.rearrange("p h n -> p (h n)"),
            )
            eng_ns["dve"] += (58 + 2 * TILE) / 0.96
        pend["m2t"] = m2t

    def stats_negmm_sqrt(pend):
        tgt, m2t = pend["p4"][1], pend["m2t"]
        std_sb = msp.tile([40, 2, TILE], f32, tag="std")
        for h in range(2):
            # tgt[32:40] -= mean^2  (PE accumulate; closes the sumsq group)
            nc.tensor.matmul(
                out=tgt[32:40, h, :], lhsT=negd, rhs=m2t[:, h, :],
                start=False, stop=True, tile_position=(0, 32),
            )
        # std = sqrt(16/15 * (E[p^2] - mean^2))
        nc.scalar.activation(
            out=std_sb[32:40, :, :].rearrange("p h n -> p (h n)"),
            in_=tgt[32:40, :, :].rearrange("p h n -> p (h n)"),
            func=AF.Sqrt, scale=16.0 / 15.0,
        )
        eng_ns["act"] += _act_cost(2 * TILE)
        pend["std_sb"] = std_sb

    def xt_fetch(t):
        x0 = t * DTILE
        tile = xp.tile([128, 2, TILE], f16, tag="xt", name=f"xt_{t}")
        nc.sync.dma_start(
            out=tile,
            in_=xt[:, x0 : x0 + DTILE].rearrange("p (h n) -> p h n", n=TILE),
        )
        return tile

    # Software pipeline with fine-grained interleaving: dtile t-1's L4,
    # preds, and stats-chain links are woven between dtile t's duo bursts.
    # Each deferred block's dependencies resolved >=2 phases earlier, so it
    # never stalls the PE FIFO head, and it gives the PE dependency-free
    # filler work during epilogue drain waits (keeps HAM warm).
    # Input tiles are prefetched 2 dtiles ahead (xp bufs=3) so L1 never
    # waits on the inbound DMA.
    xt_tiles = {0: xt_fetch(0), 1: xt_fetch(1)}
    pend = None
    for t in range(n_dt + 1):
        run = t < n_dt
        if run:
            x0 = t * DTILE
            if t + 2 < n_dt:
                xt_tiles[t + 2] = xt_fetch(t + 2)
            xt_t = xt_tiles.pop(t)
            h1, h2, h3 = {}, {}, {}
            layer_duo(1, t, xt_t, None, h1, 0)
        if pend is not None:
            l4_block(0, pend)
        if run:
            layer_duo(1, t, xt_t, None, h1, 1)
            layer_duo(1, t, xt_t, None, h1, 2)
            layer_duo(1, t, xt_t, None, h1, 3)
        if pend is not None:
            l4_block(1, pend)
        if run:
            layer_duo(2, t, None, h1, h2, 0)
            layer_duo(2, t, None, h1, h2, 1)
        if pend is not None:
            stats_sel_mean(pend)
        if run:
            layer_duo(2, t, None, h1, h2, 2)
        if pend is not None:
            stats_copy(pend)
        if run:
            layer_duo(2, t, None, h1, h2, 3)
        if pend is not None:
            # sq (gpsimd) has landed by now
            stats_sel_sq(pend)
            stats_m2(pend)
            nc.sync.dma_start(
                out=meant[:, pend["x0"] : pend["x0"] + DTILE].rearrange(
                    "p (h n) -> p h n", n=TILE
                ),
                in_=pend["mean_sb"],
            )
        if run:
            layer_duo(3, t, None, h2, h3, 0)
            layer_duo(3, t, None, h2, h3, 1)
        if pend is not None:
            stats_negmm_sqrt(pend)
        if run:
            layer_duo(3, t, None, h2, h3, 2)
            layer_duo(3, t, None, h2, h3, 3)
        if pend is not None:
            nc.sync.dma_start(
                out=stdt[:, pend["x0"] : pend["x0"] + DTILE].rearrange(
                    "p (h n) -> p h n", n=TILE
                ),
                in_=pend["std_sb"][32:40, :, :],
            )
        pend = {"x0": x0, "h3": h3, "pr": {}, "sq": {}, "p4": {}} if run else None


def build(b_core=B_CORE, num_devices=N_CORES):
    from contextlib import ExitStack

    import concourse.bacc as bacc
    import concourse.tile as tile
    from concourse import mybir

    f32 = mybir.dt.float32
    f16 = mybir.dt.float16
    nc = bacc.Bacc(
        "TRN2", target_bir_lowering=False, debug=False, num_devices=num_devices
    )
    xt = nc.dram_tensor("xt", [128, b_core], f16, kind="ExternalInput").ap()
    wr = nc.dram_tensor("wpackr", [128, WR], f16, kind="ExternalInput").ap()
    wb = nc.dram_tensor("wpackb", [128, WB], f32, kind="ExternalInput").ap()
    meant = nc.dram_tensor("meant", [8, b_core], f32, kind="ExternalOutput").ap()
    stdt = nc.dram_tensor("stdt", [8, b_core], f32, kind="ExternalOutput").ap()
    with tile.TileContext(nc) as tc:
        with ExitStack() as ctx:
            _emit(tc, ctx, xt, wr, wb, meant, stdt, b_core)
    nc.compile()
    return nc


_NC_CACHE = {}


def kernel(x, W1, b1, W2, b2, W3, b3, W4, b4):
    from concourse.bass_utils import run_bass_kernel_spmd

    key = ("full", B_CORE)
    if key not in _NC_CACHE:
        _NC_CACHE[key] = build(B_CORE, N_CORES)
    nc = _NC_CACHE[key]

    xt_per_core, wpackr, wpackb = pack_inputs(
        np.asarray(x), np.asarray(W1), np.asarray(b1), np.asarray(W2),
        np.asarray(b2), np.asarray(W3), np.asarray(b3), np.asarray(W4),
        np.asarray(b4),
    )
    in_maps = [
        {"xt": xt_per_core[c], "wpackr": wpackr, "wpackb": wpackb}
        for c in range(N_CORES)
    ]
    res = run_bass_kernel_spmd(nc, in_maps, list(range(N_CORES))).results
    mean = np.concatenate([res[c]["meant"] for c in range(N_CORES)], axis=1).T
    std = np.concatenate([res[c]["stdt"] for c in range(N_CORES)], axis=1).T
    return np.ascontiguousarray(mean), np.ascontiguousarray(std)


# revision 39
# speedup vs baseline: 1.0212x; 1.0023x over previous
"""Ensemble MLP surrogate (16 models, 32->64->64->64->8, relu) on 8 TRN2 cores.

Data-parallel over batch, weights replicated.  Feature-on-partition layout,
batch streamed as the matmul moving operand in fp16.  Per double-tile of 1024
batch elements the schedule is epilogue-bound (PSUM->SBUF bias+ReLU on the
Scalar/Vector engines), so the design centers on keeping those two engines
saturated:

  - L1 runs as 8-MM octets (two model-pairs concurrently, all 16 PE array
    tiles busy) via parity-swapped W1 packing for odd pairs; L2/L3 as 4-MM
    quads; L4 packs pair blockdiags into col-tiled preds banks.
  - PSUM (8 banks): php pool bufs=3 x [128,2,512] (6 banks; 3-deep rotation
    so matmul fills overlap drains) + p4 pool bufs=2 x [128,512] (2 banks).
    The ensemble mean / sum-of-squares selector matmuls are overlaid into
    unused partition rows (16-23 / 48-55) of the group-1 preds tiles, so the
    stats need no PSUM of their own.
  - Squares (pred^2, mean^2) run on the otherwise-idle GPSIMD engine; the
    E[p^2]-mean^2 subtraction is a -identity accumulating matmul on the PE.
  - Bias+ReLU epilogues are greedily balanced across Scalar and Vector.
  - Input DMA on sync (HWDGE), output DMA on scalar (HWDGE).
"""

import numpy as np

N_MODELS = 16
IN_DIM = 32
HID = 64
OUT_DIM = 8
BATCH = 131072
N_CORES = 8
B_CORE = BATCH // N_CORES  # 16384
TILE = 512  # PSUM bank limit on matmul out cols (fp32)
DTILE = 2 * TILE  # batch elements per pipeline step
NPAIR = N_MODELS // 2

# wpackr free-dim layout (fp16 matmul operands, 128 partitions)
OFF_W1 = 0  # [128, 8, 64]  odd pairs parity-swapped (rows b,a,b,a)
OFF_W2 = OFF_W1 + NPAIR * 64  # [128, 8, 64] canonical (a; b)
OFF_W3 = OFF_W2 + NPAIR * 64  # [128, 8, 64] odd pairs input-swapped (b; a)
OFF_W4 = OFF_W3 + NPAIR * 64  # [128, 8, 32] pair blockdiag (cols 0-15 zero)
OFF_SELP = OFF_W4 + NPAIR * 32  # [128, 8] mean/sumsq selector (1/16)
OFF_NEGD = OFF_SELP + 8  # [128, 8] -identity (rows 0-7)
WR = OFF_NEGD + 8
# wpackb free-dim layout (fp32 biases)
OFF_B1 = 0  # [128, 8]
OFF_B2 = OFF_B1 + NPAIR  # [128, 8] (parity-swapped odd pairs)
OFF_B3 = OFF_B2 + NPAIR  # [128, 8]
OFF_B4 = OFF_B3 + NPAIR  # [128, 2] (per L4 group)
WB = OFF_B4 + 2

USE_GPS_SQ = True  # pred squares on GPSIMD (else Vector)
USE_GPS_M2 = True  # mean^2 on GPSIMD (else Vector)


# epilogue op costs (ns) for greedy ACT/DVE load balancing (HW-measured)
def _act_cost(fd):
    return (315 + fd) / 1.2


def _dve_cost(fd):
    return (208 + fd) / 0.96


def pack_inputs(x, W1, b1, W2, b2, W3, b3, W4, b4, b_core=B_CORE, n_cores=N_CORES):
    """Host-side packing. Returns (xt_per_core list, wpackr fp16, wpackb f32)."""
    f32 = np.float32
    x = np.ascontiguousarray(x, dtype=f32)
    wpack = np.zeros((128, WR), f32)
    wpackb = np.zeros((128, WB), f32)

    w1v = wpack[:, OFF_W1 : OFF_W1 + NPAIR * 64].reshape(128, NPAIR, 64)
    w2v = wpack[:, OFF_W2 : OFF_W2 + NPAIR * 64].reshape(128, NPAIR, 64)
    w3v = wpack[:, OFF_W3 : OFF_W3 + NPAIR * 64].reshape(128, NPAIR, 64)
    w4v = wpack[:, OFF_W4 : OFF_W4 + NPAIR * 32].reshape(128, NPAIR, 32)
    for j in range(NPAIR):
        a, b = 2 * j, 2 * j + 1
        for k in range(4):
            # L1 row-group k: even pairs rows a,b,a,b; odd pairs b,a,b,a so the
            # swapped-col octet positions still land both outputs canonically
            if j % 2 == 0:
                w1v[32 * k : 32 * k + 32, j, :] = W1[a if k % 2 == 0 else b]
            else:
                w1v[32 * k : 32 * k + 32, j, :] = W1[b if k % 2 == 0 else a]
        w2v[0:HID, j, :] = W2[a]
        w2v[HID:128, j, :] = W2[b]
        if j % 2 == 0:  # h2 canonical input
            w3v[0:HID, j, :] = W3[a]
            w3v[HID:128, j, :] = W3[b]
        else:  # h2 swapped input (slot0 = model b)
            w3v[0:HID, j, :] = W3[b]
            w3v[HID:128, j, :] = W3[a]
        # preds land at partitions 32q+16..32q+32, freeing 32-aligned rows
        # 32q..32q+16 for the stats overlay
        w4v[0:HID, j, 16 : 16 + OUT_DIM] = W4[a]
        w4v[HID:128, j, 16 + OUT_DIM : 32] = W4[b]

    selp = wpack[:, OFF_SELP : OFF_SELP + 8]
    negd = wpack[:, OFF_NEGD : OFF_NEGD + 8]
    b4v = wpackb[:, OFF_B4 : OFF_B4 + 2]
    for q in range(4):  # pair-within-group
        for c in range(2):  # model-within-pair
            for o in range(OUT_DIM):
                p = 32 * q + 16 + 8 * c + o
                selp[p, o] = 1.0 / 16.0  # exact in fp16
                b4v[p, 0] = b4[2 * q + c, o]  # group 0: pairs 0-3
                b4v[p, 1] = b4[2 * (q + 4) + c, o]  # group 1: pairs 4-7
    for o in range(OUT_DIM):
        negd[o, o] = -1.0
    for j in range(NPAIR):
        a, b = 2 * j, 2 * j + 1
        wpackb[0:HID, OFF_B1 + j] = b1[a]
        wpackb[HID:128, OFF_B1 + j] = b1[b]
        # h2 output of odd pairs is parity-swapped (slot0 = model b)
        wpackb[0:HID, OFF_B2 + j] = b2[a if j % 2 == 0 else b]
        wpackb[HID:128, OFF_B2 + j] = b2[b if j % 2 == 0 else a]
        wpackb[0:HID, OFF_B3 + j] = b3[a]
        wpackb[HID:128, OFF_B3 + j] = b3[b]

    wpack16 = wpack.astype(np.float16)
    x16 = x.astype(np.float16)
    xt_per_core = []
    for c in range(n_cores):
        shard = x16[c * b_core : (c + 1) * b_core]  # [b_core, 32]
        xt = np.ascontiguousarray(np.tile(shard.T, (4, 1)))  # [128, b_core]
        xt_per_core.append(xt)
    return xt_per_core, wpack16, wpackb


def _emit(tc, ctx, xt, wr, wb, meant, stdt, b_core):
    import concourse.bass as bass  # noqa: F401
    from concourse import mybir

    nc = tc.nc
    f32 = mybir.dt.float32
    f16 = mybir.dt.float16
    AF = mybir.ActivationFunctionType
    ALU = mybir.AluOpType

    n_dt = b_core // DTILE

    consts = ctx.enter_context(tc.tile_pool(name="consts", bufs=1))
    xp = ctx.enter_context(tc.tile_pool(name="xp", bufs=3))
    hp = [
        ctx.enter_context(tc.tile_pool(name=f"h{i}p", bufs=8)) for i in range(3)
    ]
    prp = ctx.enter_context(tc.tile_pool(name="prp", bufs=4))
    sqp = ctx.enter_context(tc.tile_pool(name="sqp", bufs=4))
    msp = ctx.enter_context(tc.tile_pool(name="msp", bufs=2))  # mean/std/m2 sbuf
    # PSUM budget (8 banks): php 3x[128,2,512]=6 + p4p 1x[128,2,512]=2.
    # (A 4-deep single pool was tried: the extra slack lets the PE sprint
    # then idle past the HAM window -> 52% cold-clock; 3-deep backpressure
    # paces the PE and keeps it warm.)
    php = ctx.enter_context(tc.tile_pool(name="php", bufs=3, space="PSUM"))
    p4p = ctx.enter_context(tc.tile_pool(name="p4p", bufs=1, space="PSUM"))

    cw = consts.tile([128, WR], f16)
    nc.sync.dma_start(out=cw, in_=wr)
    cwb = consts.tile([128, WB], f32)
    nc.sync.dma_start(out=cwb, in_=wb)
    # tiny warmup activations: trigger both ACT table-set loads at kernel
    # start (overlapped with the const/input DMAs) instead of stalling the
    # first real epilogue ~2.6us
    warm = consts.tile([128, 2], f32, tag="warm")
    nc.scalar.activation(warm[:, 0:1], cwb[:, 0:1], AF.Relu)
    nc.scalar.activation(warm[:, 1:2], cwb[:, 0:1], AF.Sqrt)
    w1v = cw[:, OFF_W1 : OFF_W1 + NPAIR * 64].rearrange("p (j f) -> p j f", f=64)
    w2v = cw[:, OFF_W2 : OFF_W2 + NPAIR * 64].rearrange("p (j f) -> p j f", f=64)
    w3v = cw[:, OFF_W3 : OFF_W3 + NPAIR * 64].rearrange("p (j f) -> p j f", f=64)
    w4v = cw[:, OFF_W4 : OFF_W4 + NPAIR * 32].rearrange("p (j f) -> p j f", f=32)
    selp = cw[:, OFF_SELP : OFF_SELP + 8]
    negd = cw[0:8, OFF_NEGD : OFF_NEGD + 8]

    # greedy engine balancer for PSUM->SBUF epilogues
    eng_ns = {"act": 0.0, "dve": 0.0}

    def epilogue(out, in_, bias, relu):
        fd = out.free_size()
        if eng_ns["act"] + _act_cost(fd) <= eng_ns["dve"] + _dve_cost(fd):
            eng_ns["act"] += _act_cost(fd)
            nc.scalar.activation(
                out, in_, AF.Relu if relu else AF.Identity, bias=bias, scale=1.0
            )
        else:
            eng_ns["dve"] += _dve_cost(fd)
            if relu:
                nc.vector.tensor_scalar(
                    out, in_, bias, 0.0, op0=ALU.add, op1=ALU.max
                )
            else:
                nc.vector.tensor_scalar(out, in_, bias, None, op0=ALU.add)

    def l1_octet(d, xt_t, ph):
        j0, j1 = 2 * d, 2 * d + 1
        for k in range(4):
            rg = 32 * k
            c0 = 64 * (k % 2)  # even pair col half
            c1 = 64 * ((k + 1) % 2)  # odd pair col half (swapped)
            nc.tensor.matmul(
                out=ph[j0][c0 : c0 + 64, k // 2, :],
                lhsT=w1v[rg : rg + 32, j0, :],
                rhs=xt_t[rg : rg + 32, k // 2, :],
                start=True, stop=True, tile_position=(rg, c0),
            )
            nc.tensor.matmul(
                out=ph[j1][c1 : c1 + 64, k // 2, :],
                lhsT=w1v[rg : rg + 32, j1, :],
                rhs=xt_t[rg : rg + 32, k // 2, :],
                start=True, stop=True, tile_position=(rg, c1),
            )

    def mid_quads(d, wv, hin, ph):
        j0, j1 = 2 * d, 2 * d + 1
        for h in range(2):
            nc.tensor.matmul(
                out=ph[j0][0:64, h, :], lhsT=wv[0:64, j0, :],
                rhs=hin[j0][0:64, h, :], start=True, stop=True,
                tile_position=(0, 0),
            )
            nc.tensor.matmul(
                out=ph[j0][64:128, h, :], lhsT=wv[64:128, j0, :],
                rhs=hin[j0][64:128, h, :], start=True, stop=True,
                tile_position=(64, 64),
            )
            nc.tensor.matmul(
                out=ph[j1][64:128, h, :], lhsT=wv[0:64, j1, :],
                rhs=hin[j1][0:64, h, :], start=True, stop=True,
                tile_position=(0, 64),
            )
            nc.tensor.matmul(
                out=ph[j1][0:64, h, :], lhsT=wv[64:128, j1, :],
                rhs=hin[j1][64:128, h, :], start=True, stop=True,
                tile_position=(64, 0),
            )

    def layer_duo(lnum, t, xt_t, hin, hout, d):
        """One duo (2 pairs) of one layer: an 8-MM PE burst + 2 epilogues."""
        wv = (None, w2v, w3v)[lnum - 1]
        boff = (OFF_B1, OFF_B2, OFF_B3)[lnum - 1]
        j0, j1 = 2 * d, 2 * d + 1
        ph = {}
        for j in (j0, j1):
            ph[j] = php.tile(
                [128, 2, TILE], f32, tag="ph", name=f"ph{lnum}_{t}_{j}"
            )
        if lnum == 1:
            l1_octet(d, xt_t, ph)
        else:
            mid_quads(d, wv, hin, ph)
        for j in (j0, j1):
            hout[j] = hp[lnum - 1].tile(
                [128, 2, TILE], f16, tag=f"h{lnum}", name=f"h{lnum}_{t}_{j}"
            )
            epilogue(
                hout[j].rearrange("p h n -> p (h n)"),
                ph[j].rearrange("p h n -> p (h n)"),
                cwb[:, boff + j : boff + j + 1],
                relu=True,
            )

    def l4_block(g, pend):
        """Deferred L4 for group g of the previous dtile (deps long resolved)."""
        h3 = pend["h3"]
        p4g = p4p.tile([128, 2, TILE], f32, tag="p4", name=f"p4_{g}")
        pend["p4"][g] = p4g
        for d in (2 * g, 2 * g + 1):
            for h in range(2):
                for j in (2 * d, 2 * d + 1):
                    q = j % 4
                    nc.tensor.matmul(
                        out=p4g[32 * q : 32 * q + 32, h, :],
                        lhsT=w4v[:, j, :],
                        rhs=h3[j][:, h, :],
                        start=True, stop=True,
                        tile_position=(0, 32 * q),
                    )
        prt = prp.tile([128, 2, TILE], f16, tag="pr", name=f"pr_{g}")
        epilogue(
            prt.rearrange("p h n -> p (h n)"),
            p4g.rearrange("p h n -> p (h n)"),
            cwb[:, OFF_B4 + g : OFF_B4 + g + 1],
            relu=False,
        )
        pend["pr"][g] = prt
        sqt = sqp.tile([128, 2, TILE], f16, tag="sq", name=f"sq_{g}")
        if USE_GPS_SQ:
            nc.gpsimd.tensor_mul(
                sqt.rearrange("p h n -> p (h n)"),
                prt.rearrange("p h n -> p (h n)"),
                prt.rearrange("p h n -> p (h n)"),
            )
        else:
            nc.vector.tensor_mul(
                sqt.rearrange("p h n -> p (h n)"),
                prt.rearrange("p h n -> p (h n)"),
                prt.rearrange("p h n -> p (h n)"),
            )
            eng_ns["dve"] += (58 + 2 * TILE) / 0.96
        pend["sq"][g] = sqt

    def stats_sel_mean(pend):
        """Ensemble mean selector MMs, overlaid into unused partition rows
        0-7 (col group 0) of p4[1]; depends only on pr (ready early), so it
        doubles as safe PE filler mid-L2-phase."""
        pr, tgt = pend["pr"], pend["p4"][1]
        for h in range(2):
            nc.tensor.matmul(
                out=tgt[0:8, h, :], lhsT=selp, rhs=pr[0][:, h, :],
                start=True, stop=False, tile_position=(0, 0),
            )
            nc.tensor.matmul(
                out=tgt[0:8, h, :], lhsT=selp, rhs=pr[1][:, h, :],
                start=False, stop=True, tile_position=(0, 0),
            )

    def stats_sel_sq(pend):
        """E[p^2] selector MMs at partitions 32-39 (col group 1); deferred
        until the gpsimd squares have landed."""
        sq, tgt = pend["sq"], pend["p4"][1]
        for h in range(2):
            nc.tensor.matmul(
                out=tgt[32:40, h, :], lhsT=selp, rhs=sq[0][:, h, :],
                start=True, stop=False, tile_position=(0, 32),
            )
            nc.tensor.matmul(
                out=tgt[32:40, h, :], lhsT=selp, rhs=sq[1][:, h, :],
                start=False, stop=False, tile_position=(0, 32),
            )

    def stats_copy(pend):
        mean_sb = msp.tile([8, 2, TILE], f32, tag="mean")
        epilogue(
            mean_sb.rearrange("p h n -> p (h n)"),
            pend["p4"][1][0:8, :, :].rearrange("p h n -> p (h n)"),
            0.0, relu=False,
        )
        pend["mean_sb"] = mean_sb

    def stats_m2(pend):
        mean_sb = pend["mean_sb"]
        m2t = msp.tile([8, 2, TILE], f16, tag="m2")
        if USE_GPS_M2 and not pend.get("last"):
            nc.gpsimd.tensor_mul(
                m2t.rearrange("p h n -> p (h n)"),
                mean_sb.rearrange("p h n -> p (h n)"),
                mean_sb.rearrange("p h n -> p (h n)"),
            )
        else:
            nc.vector.tensor_mul(
                m2t.rearrange("p h n -> p (h n)"),
                mean_sb.rearrange("p h n -> p (h n)"),
                mean_sb.rearrange("p h n -> p (h n)"),
            )
            eng_ns["dve"] += (58 + 2 * TILE) / 0.96
        pend["m2t"] = m2t

    def stats_negmm_sqrt(pend):
        tgt, m2t = pend["p4"][1], pend["m2t"]
        std_sb = msp.tile([40, 2, TILE], f32, tag="std")
        for h in range(2):
            # tgt[32:40] -= mean^2  (PE accumulate; closes the sumsq group)
            nc.tensor.matmul(
                out=tgt[32:40, h, :], lhsT=negd, rhs=m2t[:, h, :],
                start=False, stop=True, tile_position=(0, 32),
            )
        # std = sqrt(16/15 * (E[p^2] - mean^2))
        nc.scalar.activation(
            out=std_sb[32:40, :, :].rearrange("p h n -> p (h n)"),
            in_=tgt[32:40, :, :].rearrange("p h n -> p (h n)"),
            func=AF.Sqrt, scale=16.0 / 15.0,
        )
        eng_ns["act"] += _act_cost(2 * TILE)
        pend["std_sb"] = std_sb

    def xt_fetch(t):
        x0 = t * DTILE
        tile = xp.tile([128, 2, TILE], f16, tag="xt", name=f"xt_{t}")
        nc.sync.dma_start(
            out=tile,
            in_=xt[:, x0 : x0 + DTILE].rearrange("p (h n) -> p h n", n=TILE),
        )
        return tile

    # Software pipeline with fine-grained interleaving: dtile t-1's L4,
    # preds, and stats-chain links are woven between dtile t's duo bursts.
    # Each deferred block's dependencies resolved >=2 phases earlier, so it
    # never stalls the PE FIFO head, and it gives the PE dependency-free
    # filler work during epilogue drain waits (keeps HAM warm).
    # Input tiles are prefetched 2 dtiles ahead (xp bufs=3) so L1 never
    # waits on the inbound DMA.
    xt_tiles = {0: xt_fetch(0), 1: xt_fetch(1)}
    pend = None
    for t in range(n_dt + 1):
        run = t < n_dt
        if run:
            x0 = t * DTILE
            if t + 2 < n_dt:
                xt_tiles[t + 2] = xt_fetch(t + 2)
            xt_t = xt_tiles.pop(t)
            h1, h2, h3 = {}, {}, {}
            layer_duo(1, t, xt_t, None, h1, 0)
        if pend is not None and not pend.get("g0_done"):
            l4_block(0, pend)
        if run:
            layer_duo(1, t, xt_t, None, h1, 1)
            layer_duo(1, t, xt_t, None, h1, 2)
            layer_duo(1, t, xt_t, None, h1, 3)
        if pend is not None:
            l4_block(1, pend)
        if run:
            layer_duo(2, t, None, h1, h2, 0)
            layer_duo(2, t, None, h1, h2, 1)
        if pend is not None:
            stats_sel_mean(pend)
        if run:
            layer_duo(2, t, None, h1, h2, 2)
        if pend is not None:
            stats_copy(pend)
        if run:
            layer_duo(2, t, None, h1, h2, 3)
        if pend is not None:
            # sq (gpsimd) has landed by now
            stats_sel_sq(pend)
            stats_m2(pend)
            nc.sync.dma_start(
                out=meant[:, pend["x0"] : pend["x0"] + DTILE].rearrange(
                    "p (h n) -> p h n", n=TILE
                ),
                in_=pend["mean_sb"],
            )
        if run:
            layer_duo(3, t, None, h2, h3, 0)
            layer_duo(3, t, None, h2, h3, 1)
        if pend is not None:
            stats_negmm_sqrt(pend)
        newpend = None
        if run and t == n_dt - 1:
            # tail compression: the final dtile's L4 group 0 starts right
            # after its own L3 duos 0-1 (h3 j0-j3 exist), instead of after
            # the whole dtile; its mean^2 runs on the then-idle Vector engine
            newpend = {"x0": x0, "h3": h3, "pr": {}, "sq": {}, "p4": {},
                       "last": True, "g0_done": True}
            l4_block(0, newpend)
        if run:
            layer_duo(3, t, None, h2, h3, 2)
            layer_duo(3, t, None, h2, h3, 3)
        if pend is not None:
            nc.sync.dma_start(
                out=stdt[:, pend["x0"] : pend["x0"] + DTILE].rearrange(
                    "p (h n) -> p h n", n=TILE
                ),
                in_=pend["std_sb"][32:40, :, :],
            )
        if run:
            pend = newpend if newpend is not None else {
                "x0": x0, "h3": h3, "pr": {}, "sq": {}, "p4": {}
            }
        else:
            pend = None


def build(b_core=B_CORE, num_devices=N_CORES):
    from contextlib import ExitStack

    import concourse.bacc as bacc
    import concourse.tile as tile
    from concourse import mybir

    f32 = mybir.dt.float32
    f16 = mybir.dt.float16
    nc = bacc.Bacc(
        "TRN2", target_bir_lowering=False, debug=False, num_devices=num_devices
    )
    xt = nc.dram_tensor("xt", [128, b_core], f16, kind="ExternalInput").ap()
    wr = nc.dram_tensor("wpackr", [128, WR], f16, kind="ExternalInput").ap()
    wb = nc.dram_tensor("wpackb", [128, WB], f32, kind="ExternalInput").ap()
    meant = nc.dram_tensor("meant", [8, b_core], f32, kind="ExternalOutput").ap()
    stdt = nc.dram_tensor("stdt", [8, b_core], f32, kind="ExternalOutput").ap()
    with tile.TileContext(nc) as tc:
        with ExitStack() as ctx:
            _emit(tc, ctx, xt, wr, wb, meant, stdt, b_core)
    nc.compile()
    return nc


_NC_CACHE = {}


def kernel(x, W1, b1, W2, b2, W3, b3, W4, b4):
    from concourse.bass_utils import run_bass_kernel_spmd

    key = ("full", B_CORE)
    if key not in _NC_CACHE:
        _NC_CACHE[key] = build(B_CORE, N_CORES)
    nc = _NC_CACHE[key]

    xt_per_core, wpackr, wpackb = pack_inputs(
        np.asarray(x), np.asarray(W1), np.asarray(b1), np.asarray(W2),
        np.asarray(b2), np.asarray(W3), np.asarray(b3), np.asarray(W4),
        np.asarray(b4),
    )
    in_maps = [
        {"xt": xt_per_core[c], "wpackr": wpackr, "wpackb": wpackb}
        for c in range(N_CORES)
    ]
    res = run_bass_kernel_spmd(nc, in_maps, list(range(N_CORES))).results
    mean = np.concatenate([res[c]["meant"] for c in range(N_CORES)], axis=1).T
    std = np.concatenate([res[c]["stdt"] for c in range(N_CORES)], axis=1).T
    return np.ascontiguousarray(mean), np.ascontiguousarray(std)
